# revision 4
# baseline (speedup 1.0000x reference)
"""COOTensorProduct kernel for 8 Trainium2 NeuronCores (bf16 pipeline).

Math: out[b, h] = sum_{i,j} cb[h, i*64+j] * in1[b, i] * in2[b, j]
with in1/in2 [4096, 64], cb [4096, 4096] (Clebsch-Gordan coupling for
irreps '4x0e+4x1o+4x2e+4x3o' x same -> all l3).

cb is block-structured: the 16 (l1,l2) pair couplings are square
matrices that pack block-diagonally into two 128x128 stationaries.
Per core (512 batch rows), for each group s and multiplicity pair
(u, v): rhs = g1[s,u] * g2[s,v] elementwise, psum = W_s.T @ rhs.

This version runs the whole device pipeline in bf16 (rel-err budget is
2e-2; bf16 end-to-end measures ~2e-3): 1-pass matmuls instead of the
fp32 LOW/HIGH dual pass, half the DMA bytes, 2x DVE rate. Ops are
batched 4-wide over v ([128, 2048] supertiles; PSUM tiles span 4
banks) to cut instruction/semaphore count, input DMAs ride the scalar
HWDGE ring while outputs ride the sync ring, and the PSUM->SBUF
drains are spread over scalar/gpsimd/vector.
"""

import json
import numpy as np
import ml_dtypes

BF16 = ml_dtypes.bfloat16

# ---------------------------------------------------------------- problem
B = 4096
DIM = 64
NCORES = 8
BPC = B // NCORES  # 512 batch rows per core
LMAX = 3
NMULT = 4  # multiplicity of each l in '4x0e+4x1o+4x2e+4x3o'
LS = [l for l in range(LMAX + 1) for _ in range(NMULT)]

# block-diagonal packing of the 16 (l1,l2) pair matrices into 2 stationaries
PAIRS_A = [(3, 3), (3, 2), (2, 3), (1, 1)]
PAIRS_B = [(2, 2), (1, 3), (3, 1), (1, 2), (2, 1), (0, 3), (3, 0),
           (0, 2), (2, 0), (0, 1), (1, 0), (0, 0)]

_decomp_cache = None
_nc_cache = None


def _col_start(l, u):
    return sum((2 * ll + 1) * NMULT for ll in range(l)) + u * (2 * l + 1)


def _build_decomp():
    """Index bookkeeping only (no numerics): which cb entries form the two
    stationary matrices, which in1/in2 columns feed each partition row,
    and which output row h each psum row maps to."""
    global _decomp_cache
    if _decomp_cache is not None:
        return _decomp_cache

    # replicate build_cb_matrix's row layout
    layout = {}
    idx1 = 0
    for l1 in LS:
        idx2 = 0
        for l2 in LS:
            for l3 in range(abs(l1 - l2), l1 + l2 + 1):
                layout.setdefault(l3, []).append((l1, l2, idx1 * DIM + idx2))
            idx2 += 2 * l2 + 1
        idx1 += 2 * l1 + 1
    entry_row = {}
    row = 0
    for l3 in sorted(layout):
        for (l1, l2, co) in sorted(layout[l3], key=lambda x: x[0] * LMAX + x[1]):
            entry_row[(l3, co)] = row
            row += 2 * l3 + 1
    assert row == B

    groups = []
    for pairs in (PAIRS_A, PAIRS_B):
        assert sum((2 * a + 1) * (2 * b + 1) for a, b in pairs) == 128
        c1 = np.zeros((NMULT, 128), dtype=np.int64)
        c2 = np.zeros((NMULT, 128), dtype=np.int64)
        h_of = np.zeros((NMULT, NMULT, 128), dtype=np.int64)
        w_k, w_m, w_h, w_c = [], [], [], []  # W[k,m] = cb[h, c]
        off = 0
        for (l1, l2) in pairs:
            n1, n2 = 2 * l1 + 1, 2 * l2 + 1
            kp = n1 * n2
            kk = np.arange(kp)
            m1, m2 = kk // n2, kk % n2
            for u in range(NMULT):
                c1[u, off:off + kp] = _col_start(l1, u) + m1
            for v in range(NMULT):
                c2[v, off:off + kp] = _col_start(l2, v) + m2
            mm = 0
            for l3 in range(abs(l1 - l2), l1 + l2 + 1):
                n3 = 2 * l3 + 1
                h0 = entry_row[(l3, _col_start(l1, 0) * DIM + _col_start(l2, 0))]
                km, m3m = np.meshgrid(kk, np.arange(n3), indexing="ij")
                w_k.append((off + km).ravel())
                w_m.append((off + mm + m3m).ravel())
                w_h.append((h0 + m3m).ravel())
                w_c.append(((_col_start(l1, 0) + m1[km.ravel()]) * DIM
                            + (_col_start(l2, 0) + m2[km.ravel()])))
                for u in range(NMULT):
                    for v in range(NMULT):
                        h = entry_row[(l3, _col_start(l1, u) * DIM + _col_start(l2, v))]
                        h_of[u, v, off + mm:off + mm + n3] = np.arange(h, h + n3)
                mm += n3
            off += kp
        groups.append({
            "c1": c1, "c2": c2, "h_of": h_of,
            "w_k": np.concatenate(w_k), "w_m": np.concatenate(w_m),
            "w_h": np.concatenate(w_h), "w_c": np.concatenate(w_c),
        })

    # global output row -> h map: tile t = S*16 + u*4 + v holds rows
    # t*128 + mm  ->  h_of[S][u, v, mm]
    hglob = np.zeros(32 * 128, dtype=np.int64)
    for s, g in enumerate(groups):
        for u in range(NMULT):
            for v in range(NMULT):
                t = s * 16 + u * 4 + v
                hglob[t * 128:(t + 1) * 128] = g["h_of"][u, v]
    _decomp_cache = (groups, hglob)
    return _decomp_cache


def _split_waits(bir_bytes):
    """This container's walrus build rejects >1 sync-wait per instruction
    ("Too many sync wait commands"). Hoist extra waits onto standalone
    EventSemaphore instructions on the same engine (same lowering raw
    bass wait_ge uses)."""
    bir = json.loads(bir_bytes)
    n = 0
    for fn in bir["functions"]:
        for blk in fn["blocks"]:
            out = []
            for inst in blk["instructions"]:
                si = inst.get("sync_info")
                waits = (si or {}).get("on_wait") or []
                if len(waits) > 1:
                    for w in waits[:-1]:
                        n += 1
                        out.append({
                            "debug": inst.get("debug", 0),
                            "engine": inst["engine"],
                            "ins": [], "outs": [],
                            "name": f"I-wsplit-{n}",
                            "opcode": "EventSemaphore",
                            "sync_info": {"on_update": [], "on_wait": [w]},
                        })
                    si["on_wait"] = [waits[-1]]
                out.append(inst)
            blk["instructions"] = out
    return json.dumps(bir).encode()


def _build_nc():
    """Bass program, identical on all 8 cores (SPMD; per-core data differs).

    Per core: 4 input supertiles [128, 2048] bf16 (one per (input, s),
    u/v-major), 8 DVE products ([128, 2048], in0 broadcast over v), 32
    bf16 [128x128x512] matmuls into 4-bank PSUM supertiles, 8 batched
    PSUM->SBUF bf16 copies (scalar/gpsimd/vector), 8 output DMAs (sync
    ring) with input DMAs on the scalar HWDGE ring.
    """
    global _nc_cache
    if _nc_cache is not None:
        return _nc_cache
    import concourse.bass as bass
    import concourse.mybir as mybir
    from concourse.tile import TileContext

    bf16 = mybir.dt.bfloat16
    f32 = mybir.dt.float32
    nc = bass.Bass()
    w = nc.dram_tensor("w", [2, 128, 128], bf16, kind="ExternalInput")
    g1 = nc.dram_tensor("g1", [2, 128, 4 * BPC], bf16, kind="ExternalInput")
    g2 = nc.dram_tensor("g2", [2, 128, 4 * BPC], bf16, kind="ExternalInput")
    o = nc.dram_tensor("o", [8, 128, 4 * BPC], bf16, kind="ExternalOutput")

    # columns of each [128, 2048] psum supertile drained by scalar (rest
    # go to vector, which also runs the 8 products): scalar's ACTIVATE
    # copy is 1 elem/cyc @1.2GHz, vector's fp32 tensor_copy 2/cyc @0.96.
    CSPLIT = 1280

    with TileContext(nc) as tc:
        with (
            tc.tile_pool(name="sb", bufs=1) as sb,
            tc.tile_pool(name="psum", bufs=2, space="PSUM") as psumpool,
        ):
            wt = []
            for s in range(2):
                t = sb.tile([128, 128], bf16, tag=f"w{s}", name=f"w{s}", bufs=1)
                nc.scalar.dma_start(out=t, in_=w[s, :, :])
                wt.append(t)
            g1t, g2t = [], []
            for s in range(2):
                # s=0 inputs on the scalar HWDGE ring (needed first),
                # s=1 on the gpsimd SWDGE ring so both load concurrently.
                ring = nc.scalar if s == 0 else nc.gpsimd
                t = sb.tile([128, 4 * BPC], bf16, tag=f"g1_{s}", name=f"g1_{s}",
                            bufs=1)
                ring.dma_start(out=t, in_=g1[s, :, :])
                g1t.append(t)
                t = sb.tile([128, 4 * BPC], bf16, tag=f"g2_{s}", name=f"g2_{s}",
                            bufs=1)
                ring.dma_start(out=t, in_=g2[s, :, :])
                g2t.append(t)

            for s in range(2):
                for u in range(NMULT):
                    st = s * 4 + u
                    rhs = sb.tile([128, 4 * BPC], bf16, tag="rhs", bufs=4)
                    # in0: g1 u-slice broadcast across the 4 v quarters
                    in0 = g1t[s][:, u * BPC:(u + 1) * BPC]
                    in0 = in0.unsqueeze(1).broadcast_to([128, 4, BPC])
                    in1 = g2t[s].rearrange("p (v c) -> p v c", v=4)
                    nc.vector.tensor_mul(
                        out=rhs.rearrange("p (v c) -> p v c", v=4),
                        in0=in0, in1=in1)
                    ps = psumpool.tile([128, 4 * BPC], f32, tag="ps")
                    for v in range(NMULT):
                        nc.tensor.matmul(
                            ps[:, v * BPC:(v + 1) * BPC], wt[s],
                            rhs[:, v * BPC:(v + 1) * BPC],
                            start=True, stop=True)
                    # drain psum as fp32 (cast-free copies run 2x on DVE;
                    # the output DMA does the fp32->bf16 cast in-flight)
                    ot = sb.tile([128, 4 * BPC], f32, tag="ot", bufs=3)
                    nc.scalar.copy(out=ot[:, :CSPLIT], in_=ps[:, :CSPLIT])
                    nc.vector.tensor_copy(out=ot[:, CSPLIT:], in_=ps[:, CSPLIT:])
                    nc.gpsimd.dma_start(out=o[st, :, :], in_=ot)

    orig = nc.to_json_bytes
    nc.to_json_bytes = lambda: _split_waits(orig())
    _nc_cache = nc
    return nc


def kernel(in1, in2, cb, _want_stats=False):
    from concourse.bass_utils import run_bass_kernel_spmd

    in1 = np.ascontiguousarray(np.asarray(in1, dtype=np.float32))
    in2 = np.ascontiguousarray(np.asarray(in2, dtype=np.float32))
    cb = np.asarray(cb, dtype=np.float32)
    groups, hglob = _build_decomp()

    # stationaries extracted straight from cb (no wigner math needed)
    wmat = np.zeros((2, 128, 128), dtype=np.float32)
    for s, g in enumerate(groups):
        wmat[s][g["w_k"], g["w_m"]] = cb[g["w_h"], g["w_c"]]
    wmat = wmat.astype(BF16)

    in_maps = []
    for c in range(NCORES):
        sl = slice(c * BPC, (c + 1) * BPC)
        b1 = in1[sl].T.astype(BF16)
        b2 = in2[sl].T.astype(BF16)
        gg1 = np.empty((2, 128, 4 * BPC), dtype=BF16)
        gg2 = np.empty((2, 128, 4 * BPC), dtype=BF16)
        for s, g in enumerate(groups):
            for u in range(NMULT):
                gg1[s][:, u * BPC:(u + 1) * BPC] = b1[g["c1"][u]]
                gg2[s][:, u * BPC:(u + 1) * BPC] = b2[g["c2"][u]]
        in_maps.append({"w": wmat, "g1": gg1, "g2": gg2})

    nc = _build_nc()
    import os
    trace = bool(int(os.environ.get("KERNEL_TRACE", "0")))
    res = run_bass_kernel_spmd(nc, in_maps, core_ids=list(range(NCORES)),
                               trace=trace)

    # o [8, 128, 2048]: supertile st = s*4+u, quarter v -> tile t = st*4+v
    full = np.concatenate(
        [np.asarray(r["o"], dtype=np.float32)
         .reshape(8, 128, 4, BPC).transpose(0, 2, 1, 3).reshape(32 * 128, BPC)
         for r in res.results], axis=1)
    out = np.empty((B, B), dtype=np.float32)
    out[:, hglob] = full.T
    if _want_stats:
        return out, res
    return out


if __name__ == "__main__":
    rng = np.random.default_rng(0)
    a = rng.standard_normal((B, DIM)).astype(np.float32)
    b = rng.standard_normal((B, DIM)).astype(np.float32)
    cb = np.load("/tmp/cb.npy")
    out = kernel(a, b, cb)
    outer = np.einsum("bi,bj->bij", a, b).reshape(B, -1)
    exp = outer @ cb.T
    print("rel err:", np.linalg.norm(out - exp) / np.linalg.norm(exp))


# revision 6
# speedup vs baseline: 1.3751x; 1.3751x over previous
"""COOTensorProduct kernel for 8 Trainium2 NeuronCores (bf16 pipeline).

Math: out[b, h] = sum_{i,j} cb[h, i*64+j] * in1[b, i] * in2[b, j]
with in1/in2 [4096, 64], cb [4096, 4096] (Clebsch-Gordan coupling for
irreps '4x0e+4x1o+4x2e+4x3o' x same -> all l3).

cb is block-structured: the 16 (l1,l2) pair couplings are square
matrices that pack block-diagonally into two 128x128 stationaries.
Per core (512 batch rows), for each group s and multiplicity pair
(u, v): rhs = g1[s,u] * g2[s,v] elementwise, psum = W_s.T @ rhs.

This version runs the whole device pipeline in bf16 (rel-err budget is
2e-2; bf16 end-to-end measures ~2e-3): 1-pass matmuls instead of the
fp32 LOW/HIGH dual pass, half the DMA bytes, 2x DVE rate. Ops are
batched 4-wide over v ([128, 2048] supertiles; PSUM tiles span 4
banks) to cut instruction/semaphore count, input DMAs ride the scalar
HWDGE ring while outputs ride the sync ring, and the PSUM->SBUF
drains are spread over scalar/gpsimd/vector.
"""

import json
import numpy as np
import ml_dtypes

BF16 = ml_dtypes.bfloat16

# ---------------------------------------------------------------- problem
B = 4096
DIM = 64
NCORES = 8
BPC = B // NCORES  # 512 batch rows per core
LMAX = 3
NMULT = 4  # multiplicity of each l in '4x0e+4x1o+4x2e+4x3o'
LS = [l for l in range(LMAX + 1) for _ in range(NMULT)]

# block-diagonal packing of the 16 (l1,l2) pair matrices into 2 stationaries
PAIRS_A = [(3, 3), (3, 2), (2, 3), (1, 1)]
PAIRS_B = [(2, 2), (1, 3), (3, 1), (1, 2), (2, 1), (0, 3), (3, 0),
           (0, 2), (2, 0), (0, 1), (1, 0), (0, 0)]

_decomp_cache = None
_nc_cache = None


def _col_start(l, u):
    return sum((2 * ll + 1) * NMULT for ll in range(l)) + u * (2 * l + 1)


def _build_decomp():
    """Index bookkeeping only (no numerics): which cb entries form the two
    stationary matrices, which in1/in2 columns feed each partition row,
    and which output row h each psum row maps to."""
    global _decomp_cache
    if _decomp_cache is not None:
        return _decomp_cache

    # replicate build_cb_matrix's row layout
    layout = {}
    idx1 = 0
    for l1 in LS:
        idx2 = 0
        for l2 in LS:
            for l3 in range(abs(l1 - l2), l1 + l2 + 1):
                layout.setdefault(l3, []).append((l1, l2, idx1 * DIM + idx2))
            idx2 += 2 * l2 + 1
        idx1 += 2 * l1 + 1
    entry_row = {}
    row = 0
    for l3 in sorted(layout):
        for (l1, l2, co) in sorted(layout[l3], key=lambda x: x[0] * LMAX + x[1]):
            entry_row[(l3, co)] = row
            row += 2 * l3 + 1
    assert row == B

    groups = []
    for pairs in (PAIRS_A, PAIRS_B):
        assert sum((2 * a + 1) * (2 * b + 1) for a, b in pairs) == 128
        c1 = np.zeros((NMULT, 128), dtype=np.int64)
        c2 = np.zeros((NMULT, 128), dtype=np.int64)
        h_of = np.zeros((NMULT, NMULT, 128), dtype=np.int64)
        w_k, w_m, w_h, w_c = [], [], [], []  # W[k,m] = cb[h, c]
        off = 0
        for (l1, l2) in pairs:
            n1, n2 = 2 * l1 + 1, 2 * l2 + 1
            kp = n1 * n2
            kk = np.arange(kp)
            m1, m2 = kk // n2, kk % n2
            for u in range(NMULT):
                c1[u, off:off + kp] = _col_start(l1, u) + m1
            for v in range(NMULT):
                c2[v, off:off + kp] = _col_start(l2, v) + m2
            mm = 0
            for l3 in range(abs(l1 - l2), l1 + l2 + 1):
                n3 = 2 * l3 + 1
                h0 = entry_row[(l3, _col_start(l1, 0) * DIM + _col_start(l2, 0))]
                km, m3m = np.meshgrid(kk, np.arange(n3), indexing="ij")
                w_k.append((off + km).ravel())
                w_m.append((off + mm + m3m).ravel())
                w_h.append((h0 + m3m).ravel())
                w_c.append(((_col_start(l1, 0) + m1[km.ravel()]) * DIM
                            + (_col_start(l2, 0) + m2[km.ravel()])))
                for u in range(NMULT):
                    for v in range(NMULT):
                        h = entry_row[(l3, _col_start(l1, u) * DIM + _col_start(l2, v))]
                        h_of[u, v, off + mm:off + mm + n3] = np.arange(h, h + n3)
                mm += n3
            off += kp
        groups.append({
            "c1": c1, "c2": c2, "h_of": h_of,
            "w_k": np.concatenate(w_k), "w_m": np.concatenate(w_m),
            "w_h": np.concatenate(w_h), "w_c": np.concatenate(w_c),
        })

    # global output row -> h map: tile t = S*16 + u*4 + v holds rows
    # t*128 + mm  ->  h_of[S][u, v, mm]
    hglob = np.zeros(32 * 128, dtype=np.int64)
    for s, g in enumerate(groups):
        for u in range(NMULT):
            for v in range(NMULT):
                t = s * 16 + u * 4 + v
                hglob[t * 128:(t + 1) * 128] = g["h_of"][u, v]
    _decomp_cache = (groups, hglob)
    return _decomp_cache


def _split_waits(bir_bytes):
    """This container's walrus build rejects >1 sync-wait per instruction
    ("Too many sync wait commands"). Hoist extra waits onto standalone
    EventSemaphore instructions on the same engine (same lowering raw
    bass wait_ge uses)."""
    bir = json.loads(bir_bytes)
    n = 0
    for fn in bir["functions"]:
        for blk in fn["blocks"]:
            out = []
            for inst in blk["instructions"]:
                si = inst.get("sync_info")
                waits = (si or {}).get("on_wait") or []
                if len(waits) > 1:
                    for w in waits[:-1]:
                        n += 1
                        out.append({
                            "debug": inst.get("debug", 0),
                            "engine": inst["engine"],
                            "ins": [], "outs": [],
                            "name": f"I-wsplit-{n}",
                            "opcode": "EventSemaphore",
                            "sync_info": {"on_update": [], "on_wait": [w]},
                        })
                    si["on_wait"] = [waits[-1]]
                out.append(inst)
            blk["instructions"] = out
    return json.dumps(bir).encode()


def _build_nc():
    """Raw-bass program (no Tile framework), identical on all 8 cores.

    The Tile-framework version of this kernel spent more time grinding
    ~350 EVENT_SEMAPHORE instructions (~8us per engine queue) and pool
    barriers than computing. This hand-schedules the five queues with
    ~14 semaphore waits total:

      scalar : in-DMAs (HWDGE ring A: w0 w1 g2[0] g1[0]x2), then the
               [0:CS) column drain of each psum supertile (ACTIVATE
               copy, 1 elem/cyc @1.2GHz, casts to bf16 for free)
      gpsimd : in-DMAs (SWDGE ring B: g2[1] g1[1]), then the [CS:2048)
               output DMAs with in-flight fp32->bf16 cast (SWDGE-only
               feature)
      vector : 8 products TT_k (bf16 2 elem/cyc), interleaved with the
               [CS:2048) psum drains (fp32 tensor_copy; PSUM source
               pins DVE to 1x, so vector gets the short split)
      tensor : 32 bf16 matmuls (4 per supertile into a 4-bank psum)
      sync   : the [0:CS) output DMAs (plain bf16, HWDGE)

    Buffer rings: rhs x4, psum x2 (4 banks each), ot x3 per half.
    """
    global _nc_cache
    if _nc_cache is not None:
        return _nc_cache
    from contextlib import ExitStack
    import concourse.bass as bass
    import concourse.mybir as mybir

    bf16 = mybir.dt.bfloat16
    f32 = mybir.dt.float32
    CS = 1536  # scalar-drained columns per supertile; vector gets 512
    nc = bass.Bass()
    w = nc.dram_tensor("w", [2, 128, 128], bf16, kind="ExternalInput")
    g1 = nc.dram_tensor("g1", [2, 128, 4 * BPC], bf16, kind="ExternalInput")
    g2 = nc.dram_tensor("g2", [2, 128, 4 * BPC], bf16, kind="ExternalInput")
    o = nc.dram_tensor("o", [8, 128, 4 * BPC], bf16, kind="ExternalOutput")

    with ExitStack() as st:
        ws = [st.enter_context(nc.sbuf_tensor(f"ws{i}", [128, 128], bf16))
              for i in range(2)]
        g1t = [st.enter_context(nc.sbuf_tensor(f"g1t{i}", [128, 4 * BPC], bf16))
               for i in range(2)]
        g2t = [st.enter_context(nc.sbuf_tensor(f"g2t{i}", [128, 4 * BPC], bf16))
               for i in range(2)]
        rhs = [st.enter_context(nc.sbuf_tensor(f"rhs{i}", [128, 4 * BPC], bf16))
               for i in range(4)]
        ots = [st.enter_context(nc.sbuf_tensor(f"ots{i}", [128, CS], bf16))
               for i in range(3)]
        otv = [st.enter_context(nc.sbuf_tensor(f"otv{i}", [128, 4 * BPC - CS],
                                               f32))
               for i in range(3)]
        ps = [st.enter_context(nc.psum_tensor(f"ps{i}", [128, 4 * BPC], f32))
              for i in range(2)]
        SA, SB, SV, SP, SCs, SCv, SDs, SDv = (
            st.enter_context(nc.semaphore(f"sem{i}")) for i in range(8))

        # ---- input DMAs --------------------------------------------
        # ring A (scalar HWDGE): everything group s=0 needs, w's first
        nc.scalar.dma_start(out=ws[0][:], in_=w[0, :, :]).then_inc(SA, 16)
        nc.scalar.dma_start(out=ws[1][:], in_=w[1, :, :]).then_inc(SA, 16)
        nc.scalar.dma_start(out=g2t[0][:], in_=g2[0, :, :]).then_inc(SA, 16)
        nc.scalar.dma_start(out=g1t[0][:, :2 * BPC],
                            in_=g1[0, :, :2 * BPC]).then_inc(SA, 16)
        nc.scalar.dma_start(out=g1t[0][:, 2 * BPC:],
                            in_=g1[0, :, 2 * BPC:]).then_inc(SA, 16)
        # ring B (gpsimd SWDGE): group s=1 inputs
        nc.gpsimd.dma_start(out=g2t[1][:], in_=g2[1, :, :]).then_inc(SB, 16)
        nc.gpsimd.dma_start(out=g1t[1][:], in_=g1[1, :, :]).then_inc(SB, 16)

        def tt(k):
            s, u = divmod(k, 4)
            if k == 0:
                nc.vector.wait_ge(SA, 64)    # w0 w1 g2[0] g1[0]-half1
            elif k == 2:
                nc.vector.wait_ge(SA, 80)    # g1[0]-half2
            elif k == 4:
                nc.vector.wait_ge(SB, 32)    # g2[1] g1[1]
            if k >= 4:
                nc.vector.wait_ge(SP, 4 * (k - 3))   # rhs ring reuse (x4)
            in0 = g1t[s][:, u * BPC:(u + 1) * BPC]
            in0 = in0.unsqueeze(1).broadcast_to([128, 4, BPC])
            in1 = g2t[s][:].rearrange("p (v c) -> p v c", v=4)
            nc.vector.tensor_mul(
                out=rhs[k % 4][:].rearrange("p (v c) -> p v c", v=4),
                in0=in0, in1=in1).then_inc(SV, 1)

        def copy_v(k):
            nc.vector.wait_ge(SP, 4 * (k + 1))
            if k >= 3:
                nc.vector.wait_ge(SDv, 16 * (k - 2))  # otv ring reuse (x3)
            nc.vector.tensor_copy(
                out=otv[k % 3][:], in_=ps[k % 2][:, CS:]).then_inc(SCv, 1)

        # vector queue: products early, psum drains trail two groups
        vorder = [("t", 0), ("t", 1), ("t", 2), ("t", 3), ("c", 0),
                  ("t", 4), ("c", 1), ("t", 5), ("c", 2), ("t", 6),
                  ("c", 3), ("t", 7), ("c", 4), ("c", 5), ("c", 6), ("c", 7)]
        for kind, k in vorder:
            tt(k) if kind == "t" else copy_v(k)

        # tensor queue: 4 matmuls per supertile
        for k in range(8):
            s = k // 4
            if k == 0:
                nc.tensor.wait_ge(SA, 16)    # w0
            elif k == 4:
                nc.tensor.wait_ge(SA, 32)    # w1
            nc.tensor.wait_ge(SV, k + 1)     # rhs[k] written
            if k >= 2:                       # psum ring reuse (x2)
                nc.tensor.wait_ge(SCs, k - 1)
                nc.tensor.wait_ge(SCv, k - 1)
            for v in range(NMULT):
                nc.tensor.matmul(
                    ps[k % 2][:, v * BPC:(v + 1) * BPC], ws[s][:],
                    rhs[k % 4][:, v * BPC:(v + 1) * BPC],
                    start=True, stop=True).then_inc(SP, 1)

        # scalar queue (after its in-DMA triggers): [0:CS) psum drains
        for k in range(8):
            nc.scalar.wait_ge(SP, 4 * (k + 1))
            if k >= 3:
                nc.scalar.wait_ge(SDs, 16 * (k - 2))  # ots ring reuse (x3)
            nc.scalar.copy(out=ots[k % 3][:],
                           in_=ps[k % 2][:, :CS]).then_inc(SCs, 1)

        # sync queue: bf16 output DMAs for the scalar-drained columns
        for k in range(8):
            nc.sync.wait_ge(SCs, k + 1)
            nc.sync.dma_start(out=o[k, :, :CS],
                              in_=ots[k % 3][:]).then_inc(SDs, 16)

        # gpsimd queue (after its in-DMA triggers): casting output DMAs
        for k in range(8):
            nc.gpsimd.wait_ge(SCv, k + 1)
            nc.gpsimd.dma_start(out=o[k, :, CS:],
                                in_=otv[k % 3][:]).then_inc(SDv, 16)

    orig = nc.to_json_bytes
    nc.to_json_bytes = lambda: _split_waits(orig())
    _nc_cache = nc
    return nc


def kernel(in1, in2, cb, _want_stats=False):
    from concourse.bass_utils import run_bass_kernel_spmd

    in1 = np.ascontiguousarray(np.asarray(in1, dtype=np.float32))
    in2 = np.ascontiguousarray(np.asarray(in2, dtype=np.float32))
    cb = np.asarray(cb, dtype=np.float32)
    groups, hglob = _build_decomp()

    # stationaries extracted straight from cb (no wigner math needed)
    wmat = np.zeros((2, 128, 128), dtype=np.float32)
    for s, g in enumerate(groups):
        wmat[s][g["w_k"], g["w_m"]] = cb[g["w_h"], g["w_c"]]
    wmat = wmat.astype(BF16)

    in_maps = []
    for c in range(NCORES):
        sl = slice(c * BPC, (c + 1) * BPC)
        b1 = in1[sl].T.astype(BF16)
        b2 = in2[sl].T.astype(BF16)
        gg1 = np.empty((2, 128, 4 * BPC), dtype=BF16)
        gg2 = np.empty((2, 128, 4 * BPC), dtype=BF16)
        for s, g in enumerate(groups):
            for u in range(NMULT):
                gg1[s][:, u * BPC:(u + 1) * BPC] = b1[g["c1"][u]]
                gg2[s][:, u * BPC:(u + 1) * BPC] = b2[g["c2"][u]]
        in_maps.append({"w": wmat, "g1": gg1, "g2": gg2})

    nc = _build_nc()
    import os
    trace = bool(int(os.environ.get("KERNEL_TRACE", "0")))
    res = run_bass_kernel_spmd(nc, in_maps, core_ids=list(range(NCORES)),
                               trace=trace)

    # o [8, 128, 2048]: supertile st = s*4+u, quarter v -> tile t = st*4+v
    full = np.concatenate(
        [np.asarray(r["o"], dtype=np.float32)
         .reshape(8, 128, 4, BPC).transpose(0, 2, 1, 3).reshape(32 * 128, BPC)
         for r in res.results], axis=1)
    out = np.empty((B, B), dtype=np.float32)
    out[:, hglob] = full.T
    if _want_stats:
        return out, res
    return out


if __name__ == "__main__":
    rng = np.random.default_rng(0)
    a = rng.standard_normal((B, DIM)).astype(np.float32)
    b = rng.standard_normal((B, DIM)).astype(np.float32)
    cb = np.load("/tmp/cb.npy")
    out = kernel(a, b, cb)
    outer = np.einsum("bi,bj->bij", a, b).reshape(B, -1)
    exp = outer @ cb.T
    print("rel err:", np.linalg.norm(out - exp) / np.linalg.norm(exp))


# revision 24
# speedup vs baseline: 1.7019x; 1.2376x over previous
"""COOTensorProduct kernel for 8 Trainium2 NeuronCores (raw bass, bf16).

Math: out[b, h] = sum_{i,j} cb[h, i*64+j] * in1[b, i] * in2[b, j]
with in1/in2 [4096, 64], cb [4096, 4096] (Clebsch-Gordan coupling for
irreps '4x0e+4x1o+4x2e+4x3o' x same -> all l3).

cb is block-structured: the 16 (l1,l2) pair couplings are square
matrices that pack block-diagonally into two 128x128 stationaries.
Per core (512 batch rows), for each group s and multiplicity pair
(u, v): rhs = g1[s,u] * g2[s,v] elementwise, psum = W_s.T @ rhs.

Device pipeline is bf16 end-to-end (rel-err budget 2e-2; this
measures ~4e-3): single-pass matmuls (fp32 needs a LOW/HIGH dual
pass), half the DMA bytes, 2x DVE rate.

Raw bass (no Tile framework): the Tile version spent more time on
~350 auto-generated EVENT_SEMAPHORE instructions and pool barriers
than computing. Queues:

  scalar : one packed input DMA (ring A: [g2_0|g1_0], 1MiB), an ACT
           spline-table warm-up, then the [0:CS) drain of each psum
           supertile (ACTIVATE copy, 1 elem/cyc @1.2GHz, free bf16
           cast)
  sync   : tiny [w0|w1] input DMA, then the [0:CS) output DMAs
  gpsimd : packed [g2_1|g1_1] input DMA (held until ring A's input
           lands so the s=0 wire isn't slowed), then the [CS:2048)
           output DMAs with in-flight fp32->bf16 cast (SWDGE-only)
  vector : 8 products TT_k ([128,2048] bf16, 2 elem/cyc), interleaved
           with the [CS:2048) psum drains (fp32 tensor_copy; a PSUM
           source pins DVE to 1x, so vector gets the short split)
  tensor : 32 bf16 matmuls, 4 per supertile into a 4-bank psum tile

Buffer rings: rhs x4, psum x2 (4 banks each), output tiles x3 per
half. The last supertile (k=7) splits its drain evenly between
scalar and vector and its output DMA across both rings to shorten
the tail. Matmuls bump the SP semaphore once per group of 4 (the
Cayman event-accel erratum punishes dense cross-engine sem-incs).
"""

import json
import numpy as np
import ml_dtypes

BF16 = ml_dtypes.bfloat16

# ---------------------------------------------------------------- problem
B = 4096
DIM = 64
NCORES = 8
BPC = B // NCORES  # 512 batch rows per core
LMAX = 3
NMULT = 4  # multiplicity of each l in '4x0e+4x1o+4x2e+4x3o'
LS = [l for l in range(LMAX + 1) for _ in range(NMULT)]

# block-diagonal packing of the 16 (l1,l2) pair matrices into 2 stationaries
PAIRS_A = [(3, 3), (3, 2), (2, 3), (1, 1)]
PAIRS_B = [(2, 2), (1, 3), (3, 1), (1, 2), (2, 1), (0, 3), (3, 0),
           (0, 2), (2, 0), (0, 1), (1, 0), (0, 0)]

_decomp_cache = None
_nc_cache = None


def _col_start(l, u):
    return sum((2 * ll + 1) * NMULT for ll in range(l)) + u * (2 * l + 1)


def _build_decomp():
    """Index bookkeeping only (no numerics): which cb entries form the two
    stationary matrices, which in1/in2 columns feed each partition row,
    and which output row h each psum row maps to."""
    global _decomp_cache
    if _decomp_cache is not None:
        return _decomp_cache

    # replicate build_cb_matrix's row layout
    layout = {}
    idx1 = 0
    for l1 in LS:
        idx2 = 0
        for l2 in LS:
            for l3 in range(abs(l1 - l2), l1 + l2 + 1):
                layout.setdefault(l3, []).append((l1, l2, idx1 * DIM + idx2))
            idx2 += 2 * l2 + 1
        idx1 += 2 * l1 + 1
    entry_row = {}
    row = 0
    for l3 in sorted(layout):
        for (l1, l2, co) in sorted(layout[l3], key=lambda x: x[0] * LMAX + x[1]):
            entry_row[(l3, co)] = row
            row += 2 * l3 + 1
    assert row == B

    groups = []
    for pairs in (PAIRS_A, PAIRS_B):
        assert sum((2 * a + 1) * (2 * b + 1) for a, b in pairs) == 128
        c1 = np.zeros((NMULT, 128), dtype=np.int64)
        c2 = np.zeros((NMULT, 128), dtype=np.int64)
        h_of = np.zeros((NMULT, NMULT, 128), dtype=np.int64)
        w_k, w_m, w_h, w_c = [], [], [], []  # W[k,m] = cb[h, c]
        off = 0
        for (l1, l2) in pairs:
            n1, n2 = 2 * l1 + 1, 2 * l2 + 1
            kp = n1 * n2
            kk = np.arange(kp)
            m1, m2 = kk // n2, kk % n2
            for u in range(NMULT):
                c1[u, off:off + kp] = _col_start(l1, u) + m1
            for v in range(NMULT):
                c2[v, off:off + kp] = _col_start(l2, v) + m2
            mm = 0
            for l3 in range(abs(l1 - l2), l1 + l2 + 1):
                n3 = 2 * l3 + 1
                h0 = entry_row[(l3, _col_start(l1, 0) * DIM + _col_start(l2, 0))]
                km, m3m = np.meshgrid(kk, np.arange(n3), indexing="ij")
                w_k.append((off + km).ravel())
                w_m.append((off + mm + m3m).ravel())
                w_h.append((h0 + m3m).ravel())
                w_c.append(((_col_start(l1, 0) + m1[km.ravel()]) * DIM
                            + (_col_start(l2, 0) + m2[km.ravel()])))
                for u in range(NMULT):
                    for v in range(NMULT):
                        h = entry_row[(l3, _col_start(l1, u) * DIM + _col_start(l2, v))]
                        h_of[u, v, off + mm:off + mm + n3] = np.arange(h, h + n3)
                mm += n3
            off += kp
        groups.append({
            "c1": c1, "c2": c2, "h_of": h_of,
            "w_k": np.concatenate(w_k), "w_m": np.concatenate(w_m),
            "w_h": np.concatenate(w_h), "w_c": np.concatenate(w_c),
        })

    # global output row -> h map: tile t = S*16 + u*4 + v holds rows
    # t*128 + mm  ->  h_of[S][u, v, mm]
    hglob = np.zeros(32 * 128, dtype=np.int64)
    for s, g in enumerate(groups):
        for u in range(NMULT):
            for v in range(NMULT):
                t = s * 16 + u * 4 + v
                hglob[t * 128:(t + 1) * 128] = g["h_of"][u, v]
    _decomp_cache = (groups, hglob)
    return _decomp_cache


def _split_waits(bir_bytes):
    """This container's walrus build rejects >1 sync-wait per instruction
    ("Too many sync wait commands"). Hoist extra waits onto standalone
    EventSemaphore instructions on the same engine (same lowering raw
    bass wait_ge uses)."""
    bir = json.loads(bir_bytes)
    n = 0
    for fn in bir["functions"]:
        for blk in fn["blocks"]:
            out = []
            for inst in blk["instructions"]:
                si = inst.get("sync_info")
                waits = (si or {}).get("on_wait") or []
                if len(waits) > 1:
                    for w in waits[:-1]:
                        n += 1
                        out.append({
                            "debug": inst.get("debug", 0),
                            "engine": inst["engine"],
                            "ins": [], "outs": [],
                            "name": f"I-wsplit-{n}",
                            "opcode": "EventSemaphore",
                            "sync_info": {"on_update": [], "on_wait": [w]},
                        })
                    si["on_wait"] = [waits[-1]]
                out.append(inst)
            blk["instructions"] = out
    return json.dumps(bir).encode()


CS = 1664   # scalar-drained columns per supertile; vector gets 384
CS7 = 1536  # last supertile splits evenly to shorten the tail


def _build_nc():
    global _nc_cache
    if _nc_cache is not None:
        return _nc_cache
    from contextlib import ExitStack
    import concourse.bass as bass
    import concourse.mybir as mybir

    bf16 = mybir.dt.bfloat16
    f32 = mybir.dt.float32
    nc = bass.Bass()
    # inA = [g2_0 | g1_0], inB = [w0 | w1], inC = [g2_1 | g1_1]
    inA = nc.dram_tensor("inA", [128, 8 * BPC], bf16, kind="ExternalInput")
    inB = nc.dram_tensor("inB", [128, 256], bf16, kind="ExternalInput")
    inC = nc.dram_tensor("inC", [128, 8 * BPC], bf16, kind="ExternalInput")
    # separate DRAM tensors per output ring: the sync (bf16) and gpsimd
    # (casting) DMAs otherwise interleave partial writes into the same
    # DRAM rows, which corrupts intermittently (write-combine/RMW hazard)
    o_s = nc.dram_tensor("o_s", [8, 128, CS], bf16, kind="ExternalOutput")
    o_v = nc.dram_tensor("o_v", [8, 128, 4 * BPC - CS7], bf16,
                         kind="ExternalOutput")

    with ExitStack() as st:
        tA = st.enter_context(nc.sbuf_tensor("tA", [128, 8 * BPC], bf16))
        tB = st.enter_context(nc.sbuf_tensor("tB", [128, 256], bf16))
        tC = st.enter_context(nc.sbuf_tensor("tC", [128, 8 * BPC], bf16))
        rhs = [st.enter_context(nc.sbuf_tensor(f"rhs{i}", [128, 4 * BPC], bf16))
               for i in range(4)]
        ots = [st.enter_context(nc.sbuf_tensor(f"ots{i}", [128, CS], bf16))
               for i in range(4)]
        otv = [st.enter_context(nc.sbuf_tensor(f"otv{i}", [128, 4 * BPC - CS7],
                                               f32))
               for i in range(4)]
        scr = st.enter_context(nc.sbuf_tensor("scr", [128, 16], bf16))
        ps = [st.enter_context(nc.psum_tensor(f"ps{i}", [128, 4 * BPC], f32))
              for i in range(2)]
        SA, SY, SB, SV, SP, SCs, SCv, SDs, SDv = (
            st.enter_context(nc.semaphore(f"sem{i}")) for i in range(9))

        g2t = [tA[:, :4 * BPC], tC[:, :4 * BPC]]
        g1t = [tA[:, 4 * BPC:], tC[:, 4 * BPC:]]
        ws = [tB[:, :128], tB[:, 128:]]

        # ---- input DMAs: one packed transfer per ring. The HWDGE
        # rings pay a ~0.7us completion receipt per DMA (FIFO), so
        # fewer/bigger wins. inC is held behind inA so the s=0 input
        # wire gets the full HBM rate during the ramp.
        nc.scalar.dma_start(out=tA[:, :6 * BPC],
                            in_=inA[:, :6 * BPC]).then_inc(SA, 16)
        nc.scalar.dma_start(out=tA[:, 6 * BPC:],
                            in_=inA[:, 6 * BPC:]).then_inc(SA, 16)
        nc.sync.dma_start(out=tB[:, :], in_=inB[:, :]).then_inc(SY, 16)
        nc.gpsimd.wait_ge(SY, 16)
        nc.gpsimd.dma_start(out=tC[:, :], in_=inC[:, :]).then_inc(SB, 16)


        def tt(k):
            s, u = divmod(k, 4)
            if k == 0:
                nc.vector.wait_ge(SA, 16)   # [g2_0 | g1_0 u01] chunk
            elif k == 2:
                nc.vector.wait_ge(SA, 32)   # g1_0 u23 chunk
            elif k == 4:
                nc.vector.wait_ge(SB, 16)
            if k >= 4:
                nc.vector.wait_ge(SP, 4 * (k - 3))   # rhs ring reuse (x4)
            in0 = g1t[s][:, u * BPC:(u + 1) * BPC]
            in0 = in0.unsqueeze(1).broadcast_to([128, 4, BPC])
            in1 = g2t[s].rearrange("p (v c) -> p v c", v=4)
            nc.vector.tensor_mul(
                out=rhs[k % 4][:].rearrange("p (v c) -> p v c", v=4),
                in0=in0, in1=in1).then_inc(SV, 1)

        def copy_v(k):
            cs = CS7 if k == 7 else CS
            nc.vector.wait_ge(SP, 4 * (k + 1))
            if k >= 4:
                nc.vector.wait_ge(SDv, 16 * (k - 3))  # otv ring reuse (x4)
            nc.vector.tensor_copy(
                out=otv[k % 4][:, :4 * BPC - cs],
                in_=ps[k % 2][:, cs:]).then_inc(SCv, 1)

        # warm the ACT COPY spline table during the ramp so the lazy
        # ACT_TABLE_LOAD (~1.3us) doesn't sit on the first psum drain.
        # Reads initialized SBUF only (w tile, after its DMA landed).
        nc.scalar.wait_ge(SY, 16)
        nc.scalar.copy(out=scr[:], in_=tB[:, :16])

        # vector queue: products early, psum drains trail two groups
        vorder = [("t", 0), ("t", 1), ("t", 2), ("t", 3), ("c", 0),
                  ("t", 4), ("c", 1), ("t", 5), ("c", 2), ("t", 6),
                  ("c", 3), ("t", 7), ("c", 4), ("c", 5), ("c", 6), ("c", 7)]
        for kind, k in vorder:
            tt(k) if kind == "t" else copy_v(k)

        # tensor queue: 4 matmuls per supertile; SP bumped once per
        # group (sparse cross-engine sem-incs; see erratum note above)
        for k in range(8):
            s = k // 4
            if k == 0:
                nc.tensor.wait_ge(SY, 16)    # w tile
            nc.tensor.wait_ge(SV, k + 1)     # rhs[k] written
            if k >= 2:                       # psum ring reuse (x2)
                nc.tensor.wait_ge(SCs, k - 1)
                nc.tensor.wait_ge(SCv, k - 1)
            for v in range(NMULT):
                nc.tensor.matmul(
                    ps[k % 2][:, v * BPC:(v + 1) * BPC], ws[s],
                    rhs[k % 4][:, v * BPC:(v + 1) * BPC],
                    start=True, stop=True).then_inc(SP, 1)

        # scalar queue: [0:CS) psum drains
        for k in range(8):
            cs = CS7 if k == 7 else CS
            nc.scalar.wait_ge(SP, 4 * (k + 1))
            if k >= 4:
                nc.scalar.wait_ge(SDs, 16 * (k - 3))  # ots ring reuse (x4)
            nc.scalar.copy(out=ots[k % 4][:, :cs],
                           in_=ps[k % 2][:, :cs]).then_inc(SCs, 1)

        # sync queue: bf16 output DMAs for the scalar-drained columns
        for k in range(8):
            cs = CS7 if k == 7 else CS
            nc.sync.wait_ge(SCs, k + 1)
            nc.sync.dma_start(out=o_s[k, :, :cs],
                              in_=ots[k % 4][:, :cs]).then_inc(SDs, 16)

        # gpsimd queue: casting output DMAs for the vector columns
        for k in range(8):
            cs = CS7 if k == 7 else CS
            nc.gpsimd.wait_ge(SCv, k + 1)
            nc.gpsimd.dma_start(out=o_v[k, :, :4 * BPC - cs],
                                in_=otv[k % 4][:, :4 * BPC - cs]
                                ).then_inc(SDv, 16)

    orig = nc.to_json_bytes
    nc.to_json_bytes = lambda: _split_waits(orig())
    _nc_cache = nc
    return nc


def kernel(in1, in2, cb, _want_stats=False):
    from concourse.bass_utils import run_bass_kernel_spmd

    in1 = np.ascontiguousarray(np.asarray(in1, dtype=np.float32))
    in2 = np.ascontiguousarray(np.asarray(in2, dtype=np.float32))
    cb = np.asarray(cb, dtype=np.float32)
    groups, hglob = _build_decomp()

    # stationaries extracted straight from cb (no wigner math needed)
    wmat = np.zeros((2, 128, 128), dtype=np.float32)
    for s, g in enumerate(groups):
        wmat[s][g["w_k"], g["w_m"]] = cb[g["w_h"], g["w_c"]]
    wmat = wmat.astype(BF16)

    in_maps = []
    for c in range(NCORES):
        sl = slice(c * BPC, (c + 1) * BPC)
        b1 = in1[sl].T.astype(BF16)
        b2 = in2[sl].T.astype(BF16)
        gg1 = np.empty((2, 128, 4 * BPC), dtype=BF16)
        gg2 = np.empty((2, 128, 4 * BPC), dtype=BF16)
        for s, g in enumerate(groups):
            for u in range(NMULT):
                gg1[s][:, u * BPC:(u + 1) * BPC] = b1[g["c1"][u]]
                gg2[s][:, u * BPC:(u + 1) * BPC] = b2[g["c2"][u]]
        inA = np.concatenate([gg2[0], gg1[0]], axis=1)
        inB = np.concatenate([wmat[0], wmat[1]], axis=1)
        inC = np.concatenate([gg2[1], gg1[1]], axis=1)
        in_maps.append({"inA": inA, "inB": inB, "inC": inC})

    nc = _build_nc()
    import os
    trace = bool(int(os.environ.get("KERNEL_TRACE", "0")))
    res = run_bass_kernel_spmd(nc, in_maps, core_ids=list(range(NCORES)),
                               trace=trace)

    # stitch the two output rings, then supertile st = s*4+u, quarter v
    # -> tile t = st*4+v
    cols = []
    for r in res.results:
        oc = np.empty((8, 128, 4 * BPC), dtype=np.float32)
        osr = np.asarray(r["o_s"], dtype=np.float32)
        ovr = np.asarray(r["o_v"], dtype=np.float32)
        for k in range(8):
            cs = CS7 if k == 7 else CS
            oc[k][:, :cs] = osr[k][:, :cs]
            oc[k][:, cs:] = ovr[k][:, :4 * BPC - cs]
        cols.append(oc.reshape(8, 128, 4, BPC).transpose(0, 2, 1, 3)
                    .reshape(32 * 128, BPC))
    full = np.concatenate(cols, axis=1)
    out = np.empty((B, B), dtype=np.float32)
    out[:, hglob] = full.T
    if _want_stats:
        return out, res
    return out


if __name__ == "__main__":
    rng = np.random.default_rng(0)
    a = rng.standard_normal((B, DIM)).astype(np.float32)
    b = rng.standard_normal((B, DIM)).astype(np.float32)
    cb = np.load("/tmp/cb.npy")
    out = kernel(a, b, cb)
    outer = np.einsum("bi,bj->bij", a, b).reshape(B, -1)
    exp = outer @ cb.T
    print("rel err:", np.linalg.norm(out - exp) / np.linalg.norm(exp))
